# revision 1
# baseline (speedup 1.0000x reference)
"""BottomPool (cumulative max along H) Trainium2 Bass kernel.

Full input x: (16, 256, 128, 128) fp32. out[b,c,h,w] = max_{h'<=h} x[b,c,h',w].

Strategy: data-parallel over the 4096 (b,c) planes -> 512 planes per core.
Per core, planes are mapped [partition p in 0..127] x [q in 0..3] with
plane = q*128 + p. SBUF tiles hold 8 consecutive h-rows for all 512 planes
([128, 4, 8, 128] fp32 = 2MB DMAs). The cummax is a serial chain of
[128, 4*128] DVE tensor_max ops (one per h-row), carried across tiles.
No transposes, no cross-core communication.
"""

import numpy as np

import concourse.tile as tile
from concourse import bacc, mybir
from concourse.bass_utils import run_bass_kernel_spmd

N_CORES = 8
B, C, H, W = 16, 256, 128, 128
P = 128  # SBUF partitions
PLANES_PER_CORE = (B * C) // N_CORES  # 512
HS = 8  # h-rows per SBUF tile / DMA


def build_module(planes=PLANES_PER_CORE, h=H, w=W, hs=HS, n_cores=N_CORES,
                 bufs=3):
    """Build + compile the per-core Bass module (same program on all cores)."""
    assert planes % P == 0 and h % hs == 0
    q = planes // P
    ns = h // hs
    nc = bacc.Bacc(
        "TRN2", target_bir_lowering=False, debug=False, num_devices=n_cores
    )
    x = nc.dram_tensor(
        "x", [planes, h, w], mybir.dt.float32, kind="ExternalInput"
    ).ap()
    y = nc.dram_tensor(
        "y", [planes, h, w], mybir.dt.float32, kind="ExternalOutput"
    ).ap()
    xv = x.rearrange("(q p) h w -> p q h w", p=P)
    yv = y.rearrange("(q p) h w -> p q h w", p=P)

    with tile.TileContext(nc) as tc:
        with (
            tc.tile_pool(name="pin", bufs=bufs) as pin,
            tc.tile_pool(name="pout", bufs=bufs) as pout,
        ):
            prev = None
            for s in range(ns):
                tin = pin.tile([P, q, hs, w], mybir.dt.float32)
                nc.sync.dma_start(tin[:], xv[:, :, s * hs:(s + 1) * hs, :])
                tout = pout.tile([P, q, hs, w], mybir.dt.float32)
                for hh in range(hs):
                    cur = tin[:, :, hh, :]
                    o = tout[:, :, hh, :]
                    if prev is None:
                        nc.vector.tensor_copy(o, cur)
                    else:
                        nc.vector.tensor_max(o, cur, prev)
                    prev = tout[:, :, hh, :]
                nc.sync.dma_start(yv[:, :, s * hs:(s + 1) * hs, :], tout[:])
    nc.compile()
    return nc


_NC_CACHE = {}


def _get_module():
    if "nc" not in _NC_CACHE:
        _NC_CACHE["nc"] = build_module()
    return _NC_CACHE["nc"]


def kernel(x: np.ndarray) -> np.ndarray:
    assert x.shape == (B, C, H, W), x.shape
    x = np.ascontiguousarray(np.asarray(x), dtype=np.float32)
    flat = x.reshape(B * C, H, W)
    in_maps = [
        {"x": flat[k * PLANES_PER_CORE:(k + 1) * PLANES_PER_CORE]}
        for k in range(N_CORES)
    ]
    nc = _get_module()
    res = run_bass_kernel_spmd(nc, in_maps, list(range(N_CORES)))
    out = np.concatenate([r["y"] for r in res.results], axis=0)
    return out.reshape(B, C, H, W)
